# revision 32
# baseline (speedup 1.0000x reference)
"""Chunked-causal attention with sinks on 8 TRN2 NeuronCores.

Sharding: the 64 (batch, head) pairs are split 8-per-core (data parallel on
B, tensor parallel on H). Each core runs the same Bass program over its 8
pairs x 4 chunks of 1024 tokens.

The per-core shard layout is chosen for DMA/TensorE efficiency:
  - Q, K arrive pre-transposed as bf16 [pairs, D, S]: the score matmul
    contracts over D, which must sit on SBUF partitions, and bf16 is the
    matmul compute dtype either way (the host conversion is numerically
    identical to an on-device cast). Per-partition rows are contiguous.
  - V arrives as bf16 [pairs, P, nch, T, D+1] (s = t*P + p within a chunk),
    with a ones column appended: partition-major so each partition's slice
    is one contiguous DRAM run, and the ones column makes the PV matmul
    emit the softmax denominator as output column D.
  - The output is stored partition-major bf16 [pairs, nch, P, T, D] and
    un-permuted on the host.

Per (pair, chunk) the kernel computes, entirely on-chip:
  S_T[k, q] = K @ Q^T          (TensorE, bf16; scores transposed so that the
                                PV matmul can consume exp(S_T) directly)
  P_T       = exp(S_T / sqrt(D))
  O[q, :]   = P_T^T @ [V | 1]  (TensorE; the ones column yields the softmax
                                denominator in column D of the same matmul)
  out       = O[:, :D] / (O[:, D] + exp(sink))

The exp is the throughput bottleneck of the whole kernel (ScalarE ACTIVATE
processes one 128-row column per 1.2GHz cycle + ~350 cycles/instr overhead,
so a chunk's 4608 columns cost ~5.3us vs ~4us of TensorE work). It is
therefore SPLIT across two engines:
  - ScalarE computes exact spline exp for the near-diagonal key strips
    (kt 0,1,2 plus the tiny far strips packed with them), which dominate
    the softmax for low-key-count rows where per-weight accuracy matters.
  - VectorE computes a Schraudolph bit-trick exp for far strips (kt 3,4,5):
    a single tensor_scalar emits int16(round(x*(128/ln2)*scale + B)) whose
    bit pattern IS bf16(exp(x*scale)) to within +-3.3%. Numerator and
    denominator use the same approximated p, so the error largely cancels
    in the softmax ratio (measured end-to-end rel err 0.0066 vs 0.02 tol).

The kernel is software-pipelined one chunk deep: scores+exp of chunk c are
emitted before the PV matmuls of chunk c-1, so the PE never waits on a
fresh exp and every engine gets a full chunk-period of slack.
"""

import ml_dtypes
import numpy as np

import concourse.bacc as bacc
import concourse.bass as bass
import concourse.mybir as mybir
import concourse.tile as tile
from concourse.bass_utils import run_bass_kernel_spmd

N_CORES = 8
B, S, H, D = 4, 4096, 16, 128
C = 1024                # chunk size
NCH = S // C            # chunks per sequence
PAIRS = B * H           # 64 (batch, head) pairs
PPC = PAIRS // N_CORES  # pairs per core
P = 128                 # SBUF partitions
T = C // P              # 128-row tiles per chunk
SCALE = 1.0 / float(np.sqrt(D))

# Schraudolph exp constants: int16(round(x*EXP_A + EXP_B)) viewed as bf16
# approximates exp(x) within +-3.3% (EXP_B calibrated for round-to-nearest).
EXP_A = 128.0 / float(np.log(2.0))
EXP_B = 16250.4

F32 = mybir.dt.float32
BF16 = mybir.dt.bfloat16
I16 = mybir.dt.int16


def _build_program(ppc=PPC, nch=NCH):
    s_len = nch * C
    nc = bacc.Bacc("TRN2", target_bir_lowering=False, debug=False)
    qk_d = nc.dram_tensor("qk", [ppc, D, 2, s_len], BF16, kind="ExternalInput")
    v_d = nc.dram_tensor("v", [ppc, P, nch, T, D + 1], BF16, kind="ExternalInput")
    es_d = nc.dram_tensor("esink", [P, ppc], F32, kind="ExternalInput")
    out_d = nc.dram_tensor("out", [ppc, P, nch, T, D], BF16, kind="ExternalOutput")

    with tile.TileContext(nc) as tc:
        with (
            tc.tile_pool(name="loads", bufs=4) as loads,
            tc.tile_pool(name="ptile", bufs=2) as ppool,
            tc.tile_pool(name="outs", bufs=2) as opool,
            tc.tile_pool(name="small", bufs=4) as small,
            tc.tile_pool(name="spsum", bufs=2, space="PSUM") as spsum,
            tc.tile_pool(name="opsum", bufs=2, space="PSUM") as opsum,
        ):
            pending_store = [None]

            def flush_store():
                if pending_store[0] is not None:
                    osb_prev, pair_prev, ch_prev = pending_store[0]
                    nc.sync.dma_start(
                        out=out_d[pair_prev, :, ch_prev], in_=osb_prev
                    )
                    pending_store[0] = None

            # Key-tile groups packed so each group's scores/exp span is one
            # contiguous <=1024-column region (5 exp calls instead of 8).
            # Groups 1 and 4 go to VectorE (Schraudolph exp), the rest to
            # ScalarE (exact exp) -- see module docstring. The choice puts
            # one VectorE exp very early in the chunk and one late, so the
            # two exp engines' serial chains interleave instead of stacking.
            GROUPS = [[0], [1, 7], [2, 6], [3, 5], [4]]
            VECTOR_GROUPS = (1, 4)
            WIDTH = {kt: C - P * kt for kt in range(T)}
            OFF = {}
            GSPAN = []
            for gi, g in enumerate(GROUPS):
                goff = C * gi
                w = 0
                for kt in g:
                    OFF[kt] = goff + w
                    w += WIDTH[kt]
                GSPAN.append((goff, w))
            PTW = C * (len(GROUPS) - 1) + GSPAN[-1][1]

            def emit_scores_group(gi, qtb, ktb, pt_flat):
                goff, gw = GSPAN[gi]
                st = spsum.tile([P, C], F32, tag="st")
                for kt in GROUPS[gi]:
                    c0 = kt * P
                    poff = OFF[kt] - goff  # packed col of q = c0
                    # split matmuls at PSUM bank boundaries (packed col 512)
                    spans = []
                    a = c0
                    while a < C:
                        pa = poff + (a - c0)
                        room = 512 - pa % 512
                        b_ = min(a + min(room, 512), C)
                        spans.append((a, b_, pa))
                        a = b_
                    for a, b_, pa in spans:
                        nc.tensor.matmul(
                            st[:, pa:pa + (b_ - a)],
                            ktb[:, c0:c0 + P],
                            qtb[:, a:b_],
                            start=True,
                            stop=True,
                        )
                if gi in VECTOR_GROUPS:
                    nc.vector.tensor_scalar(
                        out=pt_flat[:, goff:goff + gw].bitcast(I16),
                        in0=st[:, 0:gw],
                        scalar1=EXP_A * SCALE,
                        scalar2=EXP_B,
                        op0=mybir.AluOpType.mult,
                        op1=mybir.AluOpType.add,
                    )
                else:
                    nc.scalar.activation(
                        pt_flat[:, goff:goff + gw],
                        st[:, 0:gw],
                        mybir.ActivationFunctionType.Exp,
                        scale=SCALE,
                    )
                for kt in GROUPS[gi]:
                    # zero the strictly-upper (k > q) part of the diag block
                    nc.gpsimd.affine_select(
                        out=pt_flat[:, OFF[kt]:OFF[kt] + P],
                        in_=pt_flat[:, OFF[kt]:OFF[kt] + P],
                        compare_op=mybir.AluOpType.is_ge,
                        fill=0.0,
                        base=0,
                        channel_multiplier=-1,
                        pattern=[[1, P]],
                    )

            def emit_pv_pair(state, j):
                # One pair of query-tiles (qt = 2j, 2j+1) accumulated into
                # half-chunk PSUM tile h = j//2 (2 banks; each bank holds the
                # two query-tiles of one pair at 129-float stride -- 1032B,
                # so no matmul output crosses a bank boundary).
                pt_flat, vb, es_t, pair, ch, osb, oaccs, recs = state
                h = j // 2
                if j % 2 == 0:
                    oaccs[h] = opsum.tile(
                        [P, 2, 512], F32, tag="oacc", name="oacc"
                    )
                oacc = oaccs[h]
                jl = j % 2
                for qq in range(2):
                    qt = 2 * j + qq
                    for kt in range(qt + 1):
                        nc.tensor.matmul(
                            oacc[:, jl, 129 * qq:129 * qq + 129],
                            pt_flat[:, OFF[kt] + (qt - kt) * P:
                                    OFF[kt] + (qt - kt + 1) * P],
                            vb[:, kt, :],
                            start=(kt == 0),
                            stop=(kt == qt),
                        )

            def emit_norm_half(state, h, den=True, tt=True):
                # den/reciprocal/normalize for half-chunk h (4 query-tiles).
                # den/tt can be emitted separately so an unrelated VectorE op
                # (the group-4 Schraudolph exp) can slot between them in the
                # engine FIFO.
                pt_flat, vb, es_t, pair, ch, osb, oaccs, recs = state
                oacc = oaccs[h]
                if den:
                    dent = small.tile([P, 2, 2], F32, tag="den", name="den")
                    den_src = bass.AP(
                        tensor=oacc.tensor,
                        offset=oacc.offset + 128,
                        ap=[oacc.ap[0], [512, 2], [129, 2]],
                    )
                    nc.vector.tensor_scalar_add(dent, den_src, es_t)
                    recs[h] = small.tile([P, 2, 2], F32, tag="rec", name="rec")
                    nc.vector.reciprocal(recs[h], dent)
                if not tt:
                    return
                rec = recs[h]
                osb_4d = bass.AP(
                    tensor=osb.tensor,
                    offset=osb.offset + 4 * h * D,
                    ap=[osb.ap[0], [2 * D, 2], [D, 2], [1, D]],
                )
                o_src = bass.AP(
                    tensor=oacc.tensor,
                    offset=oacc.offset,
                    ap=[oacc.ap[0], [512, 2], [129, 2], [1, D]],
                )
                rec_b = bass.AP(
                    tensor=rec.tensor,
                    offset=rec.offset,
                    ap=[rec.ap[0], [2, 2], [1, 2], [0, D]],
                )
                nc.vector.tensor_tensor(
                    osb_4d, o_src, rec_b, mybir.AluOpType.mult
                )

            # PE warmup: the HAM clock gate holds the PE at 1.2GHz until it
            # has been busy ~3.4us. The first input DMAs take ~10us to land,
            # so burn that window on dummy matmuls over a zeroed tile --
            # chunk 0 then runs at the warm 2.4GHz clock.
            warm_src = small.tile([P, 512], BF16, tag="warm")
            nc.gpsimd.memset(warm_src, 0)
            warm_st = opsum.tile([P, 2, 512], F32, tag="oacc", name="warm_st")

            def emit_warm(n):
                for _ in range(n):
                    nc.tensor.matmul(
                        warm_st[:, 0, 0:512],
                        warm_src[:, 0:P],
                        warm_src[:, 0:512],
                        start=True,
                        stop=True,
                    )

            emit_warm(8)

            # One-chunk software pipeline, interleaved so that (a) the PE
            # fills the score-group WAR windows (group g+2 waits on group g's
            # exp releasing its rotating PSUM tile) with PV matmuls of the
            # previous chunk, and (b) the VectorE FIFO alternates
            # [normH0(c-1), exp g3(c), normH1(c-1), exp g4(c)] -- each op
            # queued right at the point its inputs become ready, so neither
            # the exps nor the normalizes head-of-line-block each other.
            pv_state = [None]

            def emit_chunk(qtb, ktb, vb, es_t, pair, ch):
                flush_store()
                pt_flat = ppool.tile([P, PTW], BF16, tag="pt")
                prev = pv_state[0]
                emit_scores_group(0, qtb, ktb, pt_flat)
                emit_scores_group(1, qtb, ktb, pt_flat)
                if prev is not None:
                    emit_pv_pair(prev, 0)
                    emit_pv_pair(prev, 1)
                    emit_norm_half(prev, 0)
                else:
                    emit_warm(3)
                emit_scores_group(2, qtb, ktb, pt_flat)
                if prev is not None:
                    emit_pv_pair(prev, 2)
                else:
                    emit_warm(3)
                emit_scores_group(3, qtb, ktb, pt_flat)
                if prev is not None:
                    emit_pv_pair(prev, 3)
                    emit_norm_half(prev, 1, tt=False)
                else:
                    emit_warm(3)
                emit_scores_group(4, qtb, ktb, pt_flat)
                if prev is not None:
                    emit_norm_half(prev, 1, den=False)
                    pending_store[0] = (prev[5], prev[3], prev[4])
                osb = opool.tile([P, T, D], BF16, tag="osb")
                pv_state[0] = (pt_flat, vb, es_t, pair, ch, osb, [None, None], [None, None])

            def dispatch_half_loads(pair, half, first=False):
                h0 = half * 2 * C
                qkb2 = loads.tile([P, 2, 2 * C], BF16, tag="qkb", name="qkb2")
                vb2 = loads.tile([P, 2, T, D + 1], BF16, tag="vb", name="vb2")
                qtb2 = qkb2[:, 0]
                ktb2 = qkb2[:, 1]
                if first:
                    # cold start: land chunk 0's inputs in dependency order
                    # -- K tile 0 and Q chunk 0 unblock the first score
                    # group, the rest follows.
                    nc.sync.dma_start(
                        out=ktb2[:, 0:P], in_=qk_d[pair, :, 1, 0:P]
                    )
                    nc.sync.dma_start(
                        out=qtb2[:, 0:512], in_=qk_d[pair, :, 0, 0:512]
                    )
                    nc.sync.dma_start(
                        out=qtb2[:, 512:C], in_=qk_d[pair, :, 0, 512:C]
                    )
                    nc.sync.dma_start(
                        out=ktb2[:, P:C], in_=qk_d[pair, :, 1, P:C]
                    )
                    nc.sync.dma_start(out=vb2[:, 0], in_=v_d[pair, :, 0])
                    nc.sync.dma_start(
                        out=qkb2[:, :, C:2 * C],
                        in_=qk_d[pair, :, :, C:2 * C],
                    )
                    nc.sync.dma_start(out=vb2[:, 1], in_=v_d[pair, :, 1])
                else:
                    nc.sync.dma_start(
                        out=qkb2, in_=qk_d[pair, :, :, h0:h0 + 2 * C]
                    )
                    nc.sync.dma_start(
                        out=vb2, in_=v_d[pair, :, 2 * half:2 * half + 2]
                    )
                return qtb2, ktb2, vb2

            # Loads are dispatched one half (2 chunks) ahead of use, so the
            # ~1.3MB per half is fully in flight before its first score
            # matmul -- pair boundaries otherwise stall the PE on DMA.
            halves = [(p, h) for p in range(ppc) for h in range(nch // 2)]
            es_all = small.tile([P, ppc], F32, tag="esink", name="esink")
            nc.sync.dma_start(out=es_all, in_=es_d[:, :])
            es_tiles = {p: es_all[:, p:p + 1] for p in range(ppc)}
            # two halves of loads in flight ahead of the consuming chunks
            # (the loads pool's 4 buffers hold exactly: the half being
            # consumed, the previous half's V still read by the lagged PV,
            # and two prefetched halves)
            pending = [dispatch_half_loads(*halves[0], first=True)]
            if len(halves) > 1:
                pending.append(dispatch_half_loads(*halves[1]))
            for idx, (pair, half) in enumerate(halves):
                if idx + 2 < len(halves):
                    pending.append(dispatch_half_loads(*halves[idx + 2]))
                qtb2, ktb2, vb2 = pending.pop(0)
                for chsub in range(2):
                    emit_chunk(
                        qtb2[:, chsub * C:(chsub + 1) * C],
                        ktb2[:, chsub * C:(chsub + 1) * C],
                        vb2[:, chsub],
                        es_tiles[pair],
                        pair,
                        2 * half + chsub,
                    )
            # drain the last chunk's PV
            last = pv_state[0]
            for j in range(4):
                emit_pv_pair(last, j)
                if j % 2 == 1:
                    emit_norm_half(last, j // 2)
            flush_store()
            pending_store[0] = (last[5], last[3], last[4])
            flush_store()

    nc.compile()
    return nc


_PROGRAM = None


def _get_program():
    global _PROGRAM
    if _PROGRAM is None:
        _PROGRAM = _build_program()
    return _PROGRAM


def _prep_in_maps(q, k, v, sinks):
    # [B,S,H,D] -> [B*H, S, D]
    qp = np.ascontiguousarray(q.transpose(0, 2, 1, 3)).reshape(PAIRS, S, D)
    kp = np.ascontiguousarray(k.transpose(0, 2, 1, 3)).reshape(PAIRS, S, D)
    vp = np.ascontiguousarray(v.transpose(0, 2, 1, 3)).reshape(PAIRS, S, D)
    # Q, K transposed to [pairs, D, S] bf16 (matmul layout/dtype), then
    # stacked [pairs, D, 2, S] so one DMA per half loads both.
    qT = np.ascontiguousarray(qp.transpose(0, 2, 1)).astype(ml_dtypes.bfloat16)
    kT = np.ascontiguousarray(kp.transpose(0, 2, 1)).astype(ml_dtypes.bfloat16)
    qkT = np.ascontiguousarray(np.stack((qT, kT), axis=2))
    # V: bf16, partition-major [pairs, P, nch, T, D+1] with a ones column
    vaug = np.empty((PAIRS, NCH, T, P, D + 1), dtype=ml_dtypes.bfloat16)
    vaug[..., :D] = vp.reshape(PAIRS, NCH, T, P, D).astype(ml_dtypes.bfloat16)
    vaug[..., D] = np.asarray(1.0, ml_dtypes.bfloat16)
    vaug = np.ascontiguousarray(vaug.transpose(0, 3, 1, 2, 4))
    es_pairs = np.tile(np.exp(sinks), B)  # es_pairs[i] = exp(sinks[i % H])

    in_maps = []
    for c in range(N_CORES):
        sl = slice(c * PPC, (c + 1) * PPC)
        # esink partition-major [P, ppc]: one DMA loads all pairs' sinks
        esb = np.repeat(es_pairs[sl][None, :], P, axis=0).astype(np.float32)
        in_maps.append(
            {"qk": qkT[sl], "v": vaug[sl], "esink": esb}
        )
    return in_maps


def kernel(q, k, v, sinks, chunk_size):
    assert int(chunk_size) == C
    q = np.asarray(q, dtype=np.float32)
    k = np.asarray(k, dtype=np.float32)
    v = np.asarray(v, dtype=np.float32)
    sinks = np.asarray(sinks, dtype=np.float32)
    assert q.shape == (B, S, H, D)

    in_maps = _prep_in_maps(q, k, v, sinks)
    nc = _get_program()
    res = run_bass_kernel_spmd(nc, in_maps, core_ids=list(range(N_CORES)))

    outp = np.concatenate([res.results[c]["out"] for c in range(N_CORES)], axis=0)
    outp = np.asarray(outp, dtype=np.float32)
    # [pairs, p, chunk, t, d] -> [pairs, s, d] (s = chunk*C + t*P + p)
    outp = outp.transpose(0, 2, 3, 1, 4).reshape(PAIRS, S, D)
    out = outp.reshape(B, H, S, D).transpose(0, 2, 1, 3)
    return np.ascontiguousarray(out)


# revision 33
# speedup vs baseline: 1.2434x; 1.2434x over previous
"""Chunked-causal attention with sinks on 8 TRN2 NeuronCores.

Sharding: the 64 (batch, head) pairs are split 8-per-core (data parallel on
B, tensor parallel on H). Each core runs the same Bass program over its 8
pairs x 4 chunks of 1024 tokens.

The per-core shard layout is chosen for DMA/TensorE efficiency:
  - Q, K arrive pre-transposed as bf16 [pairs, D, S]: the score matmul
    contracts over D, which must sit on SBUF partitions, and bf16 is the
    matmul compute dtype either way (the host conversion is numerically
    identical to an on-device cast). Per-partition rows are contiguous.
  - V arrives as bf16 [pairs, P, nch, T, D+1] (s = t*P + p within a chunk),
    with a ones column appended: partition-major so each partition's slice
    is one contiguous DRAM run, and the ones column makes the PV matmul
    emit the softmax denominator as output column D.
  - The output is stored partition-major bf16 [pairs, nch, P, T, D] and
    un-permuted on the host.

Per (pair, chunk) the kernel computes, entirely on-chip:
  S_T[k, q] = K @ Q^T          (TensorE, bf16; scores transposed so that the
                                PV matmul can consume exp(S_T) directly)
  P_T       = exp(S_T / sqrt(D))
  O[q, :]   = P_T^T @ [V | 1]  (TensorE; the ones column yields the softmax
                                denominator in column D of the same matmul)
  out       = O[:, :D] / (O[:, D] + exp(sink))

The exp is the throughput bottleneck of the whole kernel (ScalarE ACTIVATE
processes one 128-row column per 1.2GHz cycle + ~350 cycles/instr overhead,
so a chunk's 4608 columns cost ~5.3us vs ~4us of TensorE work). It is
therefore SPLIT across two engines:
  - ScalarE computes exact spline exp for the near-diagonal key strips
    (kt 0,1,2 plus the tiny far strips packed with them), which dominate
    the softmax for low-key-count rows where per-weight accuracy matters.
  - VectorE computes a Schraudolph bit-trick exp for far strips (kt 3,4,5):
    a single tensor_scalar emits int16(round(x*(128/ln2)*scale + B)) whose
    bit pattern IS bf16(exp(x*scale)) to within +-3.3%. Numerator and
    denominator use the same approximated p, so the error largely cancels
    in the softmax ratio (measured end-to-end rel err 0.0066 vs 0.02 tol).

The kernel is software-pipelined one chunk deep: scores+exp of chunk c are
emitted before the PV matmuls of chunk c-1, so the PE never waits on a
fresh exp and every engine gets a full chunk-period of slack.
"""

import ml_dtypes
import numpy as np

import concourse.bacc as bacc
import concourse.bass as bass
import concourse.mybir as mybir
import concourse.tile as tile
from concourse.bass_utils import run_bass_kernel_spmd

N_CORES = 8
B, S, H, D = 4, 4096, 16, 128
C = 1024                # chunk size
NCH = S // C            # chunks per sequence
PAIRS = B * H           # 64 (batch, head) pairs
PPC = PAIRS // N_CORES  # pairs per core
P = 128                 # SBUF partitions
T = C // P              # 128-row tiles per chunk
SCALE = 1.0 / float(np.sqrt(D))

# Schraudolph exp constants: int16(round(x*EXP_A + EXP_B)) viewed as bf16
# approximates exp(x) within +-3.3% (EXP_B calibrated for round-to-nearest).
EXP_A = 128.0 / float(np.log(2.0))
EXP_B = 16250.4

F32 = mybir.dt.float32
BF16 = mybir.dt.bfloat16
I16 = mybir.dt.int16


def _build_program(ppc=PPC, nch=NCH):
    s_len = nch * C
    nc = bacc.Bacc("TRN2", target_bir_lowering=False, debug=False)
    qk_d = nc.dram_tensor("qk", [ppc, D, 2, s_len], BF16, kind="ExternalInput")
    v_d = nc.dram_tensor("v", [ppc, P, nch, T, D + 1], BF16, kind="ExternalInput")
    es_d = nc.dram_tensor("esink", [P, ppc], F32, kind="ExternalInput")
    out_d = nc.dram_tensor("out", [ppc, P, nch, T, D], BF16, kind="ExternalOutput")

    with tile.TileContext(nc) as tc:
        with (
            tc.tile_pool(name="loads", bufs=4) as loads,
            tc.tile_pool(name="ptile", bufs=2) as ppool,
            tc.tile_pool(name="outs", bufs=2) as opool,
            tc.tile_pool(name="small", bufs=4) as small,
            tc.tile_pool(name="spsum", bufs=2, space="PSUM") as spsum,
            tc.tile_pool(name="opsum", bufs=2, space="PSUM") as opsum,
        ):
            pending_store = [None]

            def flush_store():
                if pending_store[0] is not None:
                    osb_prev, pair_prev, ch_prev = pending_store[0]
                    nc.sync.dma_start(
                        out=out_d[pair_prev, :, ch_prev], in_=osb_prev
                    )
                    pending_store[0] = None

            # Key-tile groups packed so each group's scores/exp span is one
            # contiguous <=1024-column region (5 exp calls instead of 8).
            # Groups 1 and 4 go to VectorE (Schraudolph exp), the rest to
            # ScalarE (exact exp) -- see module docstring. The choice puts
            # one VectorE exp very early in the chunk and one late, so the
            # two exp engines' serial chains interleave instead of stacking.
            GROUPS = [[0], [1, 7], [2, 6], [3, 5], [4]]
            VECTOR_GROUPS = (1, 4)
            WIDTH = {kt: C - P * kt for kt in range(T)}
            OFF = {}
            GSPAN = []
            for gi, g in enumerate(GROUPS):
                goff = C * gi
                w = 0
                for kt in g:
                    OFF[kt] = goff + w
                    w += WIDTH[kt]
                GSPAN.append((goff, w))
            PTW = C * (len(GROUPS) - 1) + GSPAN[-1][1]

            def emit_scores_group(gi, qtb, ktb, pt_flat):
                goff, gw = GSPAN[gi]
                st = spsum.tile([P, C], F32, tag="st")
                for kt in GROUPS[gi]:
                    c0 = kt * P
                    poff = OFF[kt] - goff  # packed col of q = c0
                    # split matmuls at PSUM bank boundaries (packed col 512)
                    spans = []
                    a = c0
                    while a < C:
                        pa = poff + (a - c0)
                        room = 512 - pa % 512
                        b_ = min(a + min(room, 512), C)
                        spans.append((a, b_, pa))
                        a = b_
                    for a, b_, pa in spans:
                        nc.tensor.matmul(
                            st[:, pa:pa + (b_ - a)],
                            ktb[:, c0:c0 + P],
                            qtb[:, a:b_],
                            start=True,
                            stop=True,
                        )
                if gi in VECTOR_GROUPS:
                    nc.vector.tensor_scalar(
                        out=pt_flat[:, goff:goff + gw].bitcast(I16),
                        in0=st[:, 0:gw],
                        scalar1=EXP_A * SCALE,
                        scalar2=EXP_B,
                        op0=mybir.AluOpType.mult,
                        op1=mybir.AluOpType.add,
                    )
                else:
                    nc.scalar.activation(
                        pt_flat[:, goff:goff + gw],
                        st[:, 0:gw],
                        mybir.ActivationFunctionType.Exp,
                        scale=SCALE,
                    )
                for kt in GROUPS[gi]:
                    # zero the strictly-upper (k > q) part of the diag block
                    nc.gpsimd.affine_select(
                        out=pt_flat[:, OFF[kt]:OFF[kt] + P],
                        in_=pt_flat[:, OFF[kt]:OFF[kt] + P],
                        compare_op=mybir.AluOpType.is_ge,
                        fill=0.0,
                        base=0,
                        channel_multiplier=-1,
                        pattern=[[1, P]],
                    )

            def emit_pv_pair(state, j):
                # One pair of query-tiles (qt = 2j, 2j+1) accumulated into
                # half-chunk PSUM tile h = j//2 (2 banks; each bank holds the
                # two query-tiles of one pair at 129-float stride -- 1032B,
                # so no matmul output crosses a bank boundary).
                pt_flat, vb, es_t, pair, ch, osb, oaccs, recs = state
                h = j // 2
                if j % 2 == 0:
                    oaccs[h] = opsum.tile(
                        [P, 2, 512], F32, tag="oacc", name="oacc"
                    )
                oacc = oaccs[h]
                jl = j % 2
                for qq in range(2):
                    qt = 2 * j + qq
                    for kt in range(qt + 1):
                        nc.tensor.matmul(
                            oacc[:, jl, 129 * qq:129 * qq + 129],
                            pt_flat[:, OFF[kt] + (qt - kt) * P:
                                    OFF[kt] + (qt - kt + 1) * P],
                            vb[:, kt, :],
                            start=(kt == 0),
                            stop=(kt == qt),
                        )

            def emit_norm_half(state, h, den=True, tt=True):
                # den/reciprocal/normalize for half-chunk h (4 query-tiles).
                # den/tt can be emitted separately so an unrelated VectorE op
                # (the group-4 Schraudolph exp) can slot between them in the
                # engine FIFO.
                pt_flat, vb, es_t, pair, ch, osb, oaccs, recs = state
                oacc = oaccs[h]
                if den:
                    dent = small.tile([P, 2, 2], F32, tag="den", name="den")
                    den_src = bass.AP(
                        tensor=oacc.tensor,
                        offset=oacc.offset + 128,
                        ap=[oacc.ap[0], [512, 2], [129, 2]],
                    )
                    nc.vector.tensor_scalar_add(dent, den_src, es_t)
                    recs[h] = small.tile([P, 2, 2], F32, tag="rec", name="rec")
                    nc.vector.reciprocal(recs[h], dent)
                if not tt:
                    return
                rec = recs[h]
                osb_4d = bass.AP(
                    tensor=osb.tensor,
                    offset=osb.offset + 4 * h * D,
                    ap=[osb.ap[0], [2 * D, 2], [D, 2], [1, D]],
                )
                o_src = bass.AP(
                    tensor=oacc.tensor,
                    offset=oacc.offset,
                    ap=[oacc.ap[0], [512, 2], [129, 2], [1, D]],
                )
                rec_b = bass.AP(
                    tensor=rec.tensor,
                    offset=rec.offset,
                    ap=[rec.ap[0], [2, 2], [1, 2], [0, D]],
                )
                nc.vector.tensor_tensor(
                    osb_4d, o_src, rec_b, mybir.AluOpType.mult
                )

            # PE warmup: the HAM clock gate holds the PE at 1.2GHz until it
            # has been busy ~3.4us. The first input DMAs take ~10us to land,
            # so burn that window on dummy matmuls over a zeroed tile --
            # chunk 0 then runs at the warm 2.4GHz clock.
            warm_src = small.tile([P, 512], BF16, tag="warm")
            nc.gpsimd.memset(warm_src, 0)
            warm_st = spsum.tile([P, C], F32, tag="st", name="warm_st")
            for _ in range(5):
                nc.tensor.matmul(
                    warm_st[:, 0:512],
                    warm_src[:, 0:P],
                    warm_src[:, 0:512],
                    start=True,
                    stop=True,
                )

            # One-chunk software pipeline, interleaved so that (a) the PE
            # fills the score-group WAR windows (group g+2 waits on group g's
            # exp releasing its rotating PSUM tile) with PV matmuls of the
            # previous chunk, and (b) the VectorE FIFO alternates
            # [normH0(c-1), exp g3(c), normH1(c-1), exp g4(c)] -- each op
            # queued right at the point its inputs become ready, so neither
            # the exps nor the normalizes head-of-line-block each other.
            pv_state = [None]

            def emit_chunk(qtb, ktb, vb, es_t, pair, ch):
                flush_store()
                pt_flat = ppool.tile([P, PTW], BF16, tag="pt")
                prev = pv_state[0]
                emit_scores_group(0, qtb, ktb, pt_flat)
                emit_scores_group(1, qtb, ktb, pt_flat)
                if prev is not None:
                    emit_pv_pair(prev, 0)
                    emit_pv_pair(prev, 1)
                    emit_norm_half(prev, 0)
                emit_scores_group(2, qtb, ktb, pt_flat)
                if prev is not None:
                    emit_pv_pair(prev, 2)
                emit_scores_group(3, qtb, ktb, pt_flat)
                if prev is not None:
                    emit_pv_pair(prev, 3)
                    emit_norm_half(prev, 1, tt=False)
                emit_scores_group(4, qtb, ktb, pt_flat)
                if prev is not None:
                    emit_norm_half(prev, 1, den=False)
                    pending_store[0] = (prev[5], prev[3], prev[4])
                osb = opool.tile([P, T, D], BF16, tag="osb")
                pv_state[0] = (pt_flat, vb, es_t, pair, ch, osb, [None, None], [None, None])

            def dispatch_half_loads(pair, half, first=False):
                h0 = half * 2 * C
                qkb2 = loads.tile([P, 2, 2 * C], BF16, tag="qkb", name="qkb2")
                vb2 = loads.tile([P, 2, T, D + 1], BF16, tag="vb", name="vb2")
                qtb2 = qkb2[:, 0]
                ktb2 = qkb2[:, 1]
                if first:
                    # cold start: land chunk 0's inputs in dependency order
                    # -- K tile 0 and Q chunk 0 unblock the first score
                    # group, the rest follows.
                    nc.sync.dma_start(
                        out=ktb2[:, 0:P], in_=qk_d[pair, :, 1, 0:P]
                    )
                    nc.sync.dma_start(
                        out=qtb2[:, 0:512], in_=qk_d[pair, :, 0, 0:512]
                    )
                    nc.sync.dma_start(
                        out=qtb2[:, 512:C], in_=qk_d[pair, :, 0, 512:C]
                    )
                    nc.sync.dma_start(
                        out=ktb2[:, P:C], in_=qk_d[pair, :, 1, P:C]
                    )
                    nc.sync.dma_start(out=vb2[:, 0], in_=v_d[pair, :, 0])
                    nc.sync.dma_start(
                        out=qkb2[:, :, C:2 * C],
                        in_=qk_d[pair, :, :, C:2 * C],
                    )
                    nc.sync.dma_start(out=vb2[:, 1], in_=v_d[pair, :, 1])
                else:
                    nc.sync.dma_start(
                        out=qkb2, in_=qk_d[pair, :, :, h0:h0 + 2 * C]
                    )
                    nc.sync.dma_start(
                        out=vb2, in_=v_d[pair, :, 2 * half:2 * half + 2]
                    )
                return qtb2, ktb2, vb2

            # Loads are dispatched one half (2 chunks) ahead of use, so the
            # ~1.3MB per half is fully in flight before its first score
            # matmul -- pair boundaries otherwise stall the PE on DMA.
            halves = [(p, h) for p in range(ppc) for h in range(nch // 2)]
            es_all = small.tile([P, ppc], F32, tag="esink", name="esink")
            nc.sync.dma_start(out=es_all, in_=es_d[:, :])
            es_tiles = {p: es_all[:, p:p + 1] for p in range(ppc)}
            # two halves of loads in flight ahead of the consuming chunks
            # (the loads pool's 4 buffers hold exactly: the half being
            # consumed, the previous half's V still read by the lagged PV,
            # and two prefetched halves)
            pending = [dispatch_half_loads(*halves[0], first=True)]
            if len(halves) > 1:
                pending.append(dispatch_half_loads(*halves[1]))
            for idx, (pair, half) in enumerate(halves):
                if idx + 2 < len(halves):
                    pending.append(dispatch_half_loads(*halves[idx + 2]))
                qtb2, ktb2, vb2 = pending.pop(0)
                for chsub in range(2):
                    emit_chunk(
                        qtb2[:, chsub * C:(chsub + 1) * C],
                        ktb2[:, chsub * C:(chsub + 1) * C],
                        vb2[:, chsub],
                        es_tiles[pair],
                        pair,
                        2 * half + chsub,
                    )
            # drain the last chunk's PV
            last = pv_state[0]
            for j in range(4):
                emit_pv_pair(last, j)
                if j % 2 == 1:
                    emit_norm_half(last, j // 2)
            flush_store()
            pending_store[0] = (last[5], last[3], last[4])
            flush_store()

    nc.compile()
    return nc


_PROGRAM = None


def _get_program():
    global _PROGRAM
    if _PROGRAM is None:
        _PROGRAM = _build_program()
    return _PROGRAM


def _prep_in_maps(q, k, v, sinks):
    # [B,S,H,D] -> [B*H, S, D]
    qp = np.ascontiguousarray(q.transpose(0, 2, 1, 3)).reshape(PAIRS, S, D)
    kp = np.ascontiguousarray(k.transpose(0, 2, 1, 3)).reshape(PAIRS, S, D)
    vp = np.ascontiguousarray(v.transpose(0, 2, 1, 3)).reshape(PAIRS, S, D)
    # Q, K transposed to [pairs, D, S] bf16 (matmul layout/dtype), then
    # stacked [pairs, D, 2, S] so one DMA per half loads both.
    qT = np.ascontiguousarray(qp.transpose(0, 2, 1)).astype(ml_dtypes.bfloat16)
    kT = np.ascontiguousarray(kp.transpose(0, 2, 1)).astype(ml_dtypes.bfloat16)
    qkT = np.ascontiguousarray(np.stack((qT, kT), axis=2))
    # V: bf16, partition-major [pairs, P, nch, T, D+1] with a ones column
    vaug = np.empty((PAIRS, NCH, T, P, D + 1), dtype=ml_dtypes.bfloat16)
    vaug[..., :D] = vp.reshape(PAIRS, NCH, T, P, D).astype(ml_dtypes.bfloat16)
    vaug[..., D] = np.asarray(1.0, ml_dtypes.bfloat16)
    vaug = np.ascontiguousarray(vaug.transpose(0, 3, 1, 2, 4))
    es_pairs = np.tile(np.exp(sinks), B)  # es_pairs[i] = exp(sinks[i % H])

    in_maps = []
    for c in range(N_CORES):
        sl = slice(c * PPC, (c + 1) * PPC)
        # esink partition-major [P, ppc]: one DMA loads all pairs' sinks
        esb = np.repeat(es_pairs[sl][None, :], P, axis=0).astype(np.float32)
        in_maps.append(
            {"qk": qkT[sl], "v": vaug[sl], "esink": esb}
        )
    return in_maps


def kernel(q, k, v, sinks, chunk_size):
    assert int(chunk_size) == C
    q = np.asarray(q, dtype=np.float32)
    k = np.asarray(k, dtype=np.float32)
    v = np.asarray(v, dtype=np.float32)
    sinks = np.asarray(sinks, dtype=np.float32)
    assert q.shape == (B, S, H, D)

    in_maps = _prep_in_maps(q, k, v, sinks)
    nc = _get_program()
    res = run_bass_kernel_spmd(nc, in_maps, core_ids=list(range(N_CORES)))

    outp = np.concatenate([res.results[c]["out"] for c in range(N_CORES)], axis=0)
    outp = np.asarray(outp, dtype=np.float32)
    # [pairs, p, chunk, t, d] -> [pairs, s, d] (s = chunk*C + t*P + p)
    outp = outp.transpose(0, 2, 3, 1, 4).reshape(PAIRS, S, D)
    out = outp.reshape(B, H, S, D).transpose(0, 2, 1, 3)
    return np.ascontiguousarray(out)
